# revision 16
# baseline (speedup 1.0000x reference)
"""Trainium2 Bass kernel for nn_CasamentoMult (Casamento multivariate loss).

Math: with SIG = 1/sqrt(2*pi), the reference loss collapses to

    result = exp(-lsp) * ( D + (S1 + S2 - S3)/2 )

where D = N-2 and, with g(t) = exp(-pi*t^2):
    S1 = sum_k g(q_k),  q_k = y[k+1]-y[k]          (k in [0, D))
    S2 = sum_k g(p_k),  p_k = d[k+1]-d[k]
    S3 = 2*U - g(u_0) - g(u_D) + sum_k [g(v_k) + g(w_k)]
         u_j = d[j]-y[j],  U = sum_{j=0}^{D} g(u_j)
         v_k = d[k+1]-y[k],  w_k = d[k]-y[k+1]
    lsp = 0.5*D*(log(2*pi) + 2*log(SIG))  (~0, kept for bit-faithfulness)

so S1 + S2 - S3 = QP - VW - 2*U + g(u_0) + g(u_D) with QP = sum g(q)+g(p),
VW = sum g(v)+g(w).  The three sums need separate accumulators.

Device strategy (per core, 8 cores, feature-parallel over k):
  - inputs downcast to fp16 on host; each core gets overlapped tiles
    d,y of [128, 3907] (row r holds x[k0 .. k0+3906], k0 = cL + 3906*r)
  - DVE computes the five diff streams with fp16 tensor_sub at 2x perf
    mode into one [128, 5*3906] buffer, layout [u | v | w | q | p]
  - ACT evaluates exp(-pi t^2) = (sqrt(pi)/2)*DerivErf(sqrt(pi)*t) at
    ~1.0 cycle/elem: per column-chunk one activation over the {v,w}
    pair and one over {q,p} (3-D APs, accum_out), plus one full-width
    activation for u at the end; 11 activations, 11 accumulator columns
  - the accumulator DMA is issued but NOT waited on: the walrus NEFF
    postamble (~8us of barriers + semaphore-file clears) dwarfs the
    ~2us HBM write receipt, so the data lands long before the host can
    observe completion; out_sem is never waited on or cleared
  - fp16 rounding of the inputs perturbs each gaussian by O(1e-3) with
    near-zero bias; the checked tolerance is 2e-2 relative on a ~3e6
    result, so this is ~4 orders of magnitude inside budget.
Host finishes the 256-element tail and the u-stream endpoints in f64.
"""

import math
import numpy as np

ROWS = 128
COLS = 3906
W = COLS + 1          # overlapped row width (shift-by-1 access)
L = ROWS * COLS       # per-core elements: 499,968
NCORES = 8
N = 4000002
D = N - 2
SIG = 0.3989422804014327
SQRT_PI = math.sqrt(math.pi)

# column-chunk bounds over [0, COLS]; even starts keep fp16 subs 4B-aligned;
# first chunk small so ACT starts as early as the DMA pipeline allows
BOUNDS = [0, 512, 1536, 3906]
NCH = len(BOUNDS) - 1
USPLIT = 1954         # u stream computed as two big subs (even start)
NACC = 2 * NCH + 1    # accT columns: (vw, qp) per chunk + final u

_cached = {}


def _build_program():
    """Hand-scheduled raw-bass program (no TileContext): full-width SBUF
    tensors, forward-RAW hazards only, handled with per-chunk DMA
    semaphores and one DVE->ACT semaphore."""
    import concourse.bass as bass
    import concourse.mybir as mybir

    f32 = mybir.dt.float32
    f16 = mybir.dt.float16
    DERF = mybir.ActivationFunctionType.Derivative_Erf
    nc = bass.Bass("TRN2", target_bir_lowering=False, debug=False,
                   num_devices=NCORES)
    d_ins, y_ins = [], []
    for j in range(NCH):
        a, e = BOUNDS[j], BOUNDS[j + 1]
        cw = e - a + 1        # chunks overlap by one column
        d_ins.append(nc.declare_dram_parameter(f"d{j}", [ROWS, cw], f16,
                                               isOutput=False))
        y_ins.append(nc.declare_dram_parameter(f"y{j}", [ROWS, cw], f16,
                                               isOutput=False))
    acc_out = nc.declare_dram_parameter("acc", [ROWS, NACC], f32,
                                        isOutput=True)

    from contextlib import ExitStack
    with ExitStack() as st:
        # one DMA-completion semaphore per chunk: its 32 increments can
        # only come from that chunk's two DMAs, so wait_ge(32) is exact
        dsem = [st.enter_context(nc.semaphore(f"dsem{j}"))
                for j in range(NCH)]
        v_sem = st.enter_context(nc.semaphore("v_sem"))
        out_sem = st.enter_context(nc.semaphore("out_sem"))
        dt = st.enter_context(nc.sbuf_tensor("dt", [ROWS, W], f16))
        yt = st.enter_context(nc.sbuf_tensor("yt", [ROWS, W], f16))
        df = st.enter_context(nc.sbuf_tensor("df", [ROWS, 5 * COLS], f16))
        swidth = max(COLS, 2 * max(BOUNDS[j + 1] - BOUNDS[j]
                                   for j in range(NCH)))
        sink = st.enter_context(nc.sbuf_tensor("sink", [ROWS, swidth], f16))
        accT = st.enter_context(nc.sbuf_tensor("accT", [ROWS, NACC], f32))

        # chunk 0 issued pre-Block on the two HWDGE rings so its data is
        # in flight while the Block-entry handshake runs
        e0 = BOUNDS[1]
        nc.sync.dma_start(dt[:, 0:e0 + 1], d_ins[0][:, :]) \
            .then_inc(dsem[0], 16)
        nc.scalar.dma_start(yt[:, 0:e0 + 1], y_ins[0][:, :]) \
            .then_inc(dsem[0], 16)

        block = st.enter_context(nc.Block())

        @block.sync
        def _(sync):
            # remaining d chunks on the SP HWDGE ring
            for j in range(1, NCH):
                a, e = BOUNDS[j], BOUNDS[j + 1]
                sync.dma_start(dt[:, a:e + 1], d_ins[j][:, :]) \
                    .then_inc(dsem[j], 16)

        @block.gpsimd
        def _(gpsimd):
            # remaining y chunks on the SWDGE ring, in parallel with d
            for j in range(1, NCH):
                a, e = BOUNDS[j], BOUNDS[j + 1]
                gpsimd.dma_start(yt[:, a:e + 1], y_ins[j][:, :]) \
                      .then_inc(dsem[j], 16)

        @block.vector
        def _(vector):
            for j in range(NCH):
                a, e = BOUNDS[j], BOUNDS[j + 1]
                vector.wait_ge(dsem[j], 32)
                # stream layout in df: [u | v | w | q | p]
                vector.tensor_sub(df[:, COLS + a:COLS + e],
                                  dt[:, a + 1:e + 1], yt[:, a:e]) \
                      .then_inc(v_sem, 1)
                vector.tensor_sub(df[:, 2 * COLS + a:2 * COLS + e],
                                  dt[:, a:e], yt[:, a + 1:e + 1]) \
                      .then_inc(v_sem, 1)
                vector.tensor_sub(df[:, 3 * COLS + a:3 * COLS + e],
                                  yt[:, a + 1:e + 1], yt[:, a:e]) \
                      .then_inc(v_sem, 1)
                vector.tensor_sub(df[:, 4 * COLS + a:4 * COLS + e],
                                  dt[:, a + 1:e + 1], dt[:, a:e]) \
                      .then_inc(v_sem, 1)
            # u as two big subs once everything is resident
            vector.tensor_sub(df[:, 0:USPLIT], dt[:, 0:USPLIT],
                              yt[:, 0:USPLIT]).then_inc(v_sem, 1)
            vector.tensor_sub(df[:, USPLIT:COLS], dt[:, USPLIT:COLS],
                              yt[:, USPLIT:COLS]).then_inc(v_sem, 1)

        @block.scalar
        def _(scalar):
            # warmup activation hoists the ~1.3us erf_derivative table
            # load off the critical path (garbage in, output discarded)
            scalar.activation(sink[:, 0:1], accT[:, 0:1], DERF,
                              bias=0.0, scale=SQRT_PI)

            def pair_act(base, a, cw, col):
                in_ap = bass.AP(df, base * COLS + a,
                                [[5 * COLS, ROWS], [COLS, 2], [1, cw]])
                out_ap = bass.AP(sink, 0,
                                 [[swidth, ROWS], [cw, 2], [1, cw]])
                scalar.activation(out_ap, in_ap, DERF, bias=0.0,
                                  scale=SQRT_PI,
                                  accum_out=accT[:, col:col + 1])

            for j in range(NCH):
                a, e = BOUNDS[j], BOUNDS[j + 1]
                cw = e - a
                # v,w ready after 2 subs; q,p after 4
                scalar.wait_ge(v_sem, 4 * j + 2)
                pair_act(1, a, cw, 2 * j)          # {v, w}
                scalar.wait_ge(v_sem, 4 * j + 4)
                pair_act(3, a, cw, 2 * j + 1)      # {q, p}
            # u: one full-width activation at the end
            scalar.wait_ge(v_sem, 4 * NCH + 2)
            scalar.activation(sink[:, 0:COLS], df[:, 0:COLS], DERF,
                              bias=0.0, scale=SQRT_PI,
                              accum_out=accT[:, NACC - 1:NACC])
            # flush the ACT datapath so the last accum lands in SBUF
            # before the DMA below reads accT
            scalar.drain()
            scalar.dma_start(acc_out[:, :], accT[:, :]).then_inc(out_sem, 16)
            # no wait on out_sem: the NEFF postamble outlasts the HBM
            # write receipt by 4x.  out_sem is never waited on, so its
            # monotonic growth across executions is harmless.
            # v_sem's final value proves all dsem increments delivered.
            for s in dsem:
                scalar.sem_clear(s)
            scalar.sem_clear(v_sem)

    return nc


def _overlap_tiles(x16):
    """[N] f16 -> per-core list of per-chunk contiguous [ROWS, cw+1]
    arrays."""
    sv = x16.strides[0]
    out = []
    for c in range(NCORES):
        base = x16[c * L:]
        m = np.lib.stride_tricks.as_strided(
            base, shape=(ROWS, W), strides=(COLS * sv, sv))
        chunks = []
        for j in range(NCH):
            a, e = BOUNDS[j], BOUNDS[j + 1]
            chunks.append(np.ascontiguousarray(m[:, a:e + 1]))
        out.append(chunks)
    return out


def make_in_maps(d, y):
    """Build the per-core input dicts from full fp32 d, y."""
    d16 = np.asarray(d, dtype=np.float16)
    y16 = np.asarray(y, dtype=np.float16)
    dts = _overlap_tiles(d16)
    yts = _overlap_tiles(y16)
    in_maps = []
    for c in range(NCORES):
        m = {}
        for j in range(NCH):
            m[f"d{j}"] = dts[c][j]
            m[f"y{j}"] = yts[c][j]
        in_maps.append(m)
    return in_maps


def _g64(t):
    t = np.asarray(t, dtype=np.float64)
    return np.exp(-np.pi * t * t)


def kernel(d, y):
    from concourse.bass_utils import run_bass_kernel_spmd

    d = np.ascontiguousarray(np.asarray(d, dtype=np.float32))
    y = np.ascontiguousarray(np.asarray(y, dtype=np.float32))

    if "nc" not in _cached:
        _cached["nc"] = _build_program()
    nc = _cached["nc"]

    in_maps = make_in_maps(d, y)
    if "warm" not in _cached:
        # first execution may see stale semaphore state left on the
        # device by other programs; it self-clears at its tail, so run
        # once and discard
        run_bass_kernel_spmd(nc, in_maps, list(range(NCORES)))
        _cached["warm"] = True
    res = run_bass_kernel_spmd(nc, in_maps, list(range(NCORES))).results

    # Device partial sums of DerivErf(sqrt(pi)*t) = (2/sqrt(pi)) g(t).
    # accT columns: [vw_0, qp_0, ..., vw_{NCH-1}, qp_{NCH-1}, u]
    acc = np.stack([r["acc"] for r in res]).astype(np.float64)  # [8,128,NACC]
    cols = acc.sum(axis=(0, 1)) * (SQRT_PI / 2.0)
    VW_dev = cols[0:2 * NCH:2].sum()
    QP_dev = cols[1:2 * NCH:2].sum()
    U_dev = cols[2 * NCH]

    d64 = d.astype(np.float64)
    y64 = y.astype(np.float64)
    cov = NCORES * L                                  # 3,999,744

    # tails in f64: u over j in [cov, D], others over k in [cov, D)
    jt = np.arange(cov, D + 1)
    U = U_dev + _g64(d64[jt] - y64[jt]).sum()
    kt = np.arange(cov, D)
    VW = VW_dev + _g64(d64[kt + 1] - y64[kt]).sum() \
        + _g64(d64[kt] - y64[kt + 1]).sum()
    QP = QP_dev + _g64(d64[kt + 1] - d64[kt]).sum() \
        + _g64(y64[kt + 1] - y64[kt]).sum()

    u0 = _g64(d64[0] - y64[0])
    uD = _g64(d64[D] - y64[D])
    # S1 + S2 - S3 = QP - VW - 2U + u0 + uD
    s12m3 = QP - VW - 2.0 * U + u0 + uD

    lsp32 = np.float32(0.5 * D * (math.log(2.0 * math.pi)
                                  + 2.0 * math.log(SIG)))
    total = math.exp(-float(lsp32)) * (D + s12m3 / 2.0)
    return np.array(total, dtype=np.float32)


# revision 17
# speedup vs baseline: 1.0572x; 1.0572x over previous
"""Trainium2 Bass kernel for nn_CasamentoMult (Casamento multivariate loss).

Math: with SIG = 1/sqrt(2*pi), the reference loss collapses to

    result = exp(-lsp) * ( D + (S1 + S2 - S3)/2 )

where D = N-2 and, with g(t) = exp(-pi*t^2):
    S1 = sum_k g(q_k),  q_k = y[k+1]-y[k]          (k in [0, D))
    S2 = sum_k g(p_k),  p_k = d[k+1]-d[k]
    S3 = 2*U - g(u_0) - g(u_D) + sum_k [g(v_k) + g(w_k)]
         u_j = d[j]-y[j],  U = sum_{j=0}^{D} g(u_j)
         v_k = d[k+1]-y[k],  w_k = d[k]-y[k+1]
    lsp = 0.5*D*(log(2*pi) + 2*log(SIG))  (~0, kept for bit-faithfulness)

so S1 + S2 - S3 = QP - VW - 2*U + g(u_0) + g(u_D) with QP = sum g(q)+g(p),
VW = sum g(v)+g(w).  The three sums need separate accumulators.

Device strategy (per core, 8 cores, feature-parallel over k):
  - inputs downcast to fp16 on host; each core gets overlapped tiles
    d,y of [128, 3907] (row r holds x[k0 .. k0+3906], k0 = cL + 3906*r)
  - DVE computes the five diff streams with fp16 tensor_sub at 2x perf
    mode into one [128, 5*3906] buffer, layout [u | v | w | q | p]
  - ACT evaluates exp(-pi t^2) = (sqrt(pi)/2)*DerivErf(sqrt(pi)*t) at
    ~1.0 cycle/elem: per column-chunk one activation over the {v,w}
    pair and one over {q,p} (3-D APs, accum_out), plus one full-width
    activation for u at the end; 11 activations, 11 accumulator columns
  - the accumulator DMA is issued but NOT waited on: the walrus NEFF
    postamble (~8us of barriers + semaphore-file clears) dwarfs the
    ~2us HBM write receipt, so the data lands long before the host can
    observe completion; out_sem is never waited on or cleared
  - fp16 rounding of the inputs perturbs each gaussian by O(1e-3) with
    near-zero bias; the checked tolerance is 2e-2 relative on a ~3e6
    result, so this is ~4 orders of magnitude inside budget.
Host finishes the 256-element tail and the u-stream endpoints in f64.
"""

import math
import numpy as np

ROWS = 128
COLS = 3906
W = COLS + 1          # overlapped row width (shift-by-1 access)
L = ROWS * COLS       # per-core elements: 499,968
NCORES = 8
N = 4000002
D = N - 2
SIG = 0.3989422804014327
SQRT_PI = math.sqrt(math.pi)

# column-chunk bounds over [0, COLS]; even starts keep fp16 subs 4B-aligned;
# first chunk small so ACT starts as early as the DMA pipeline allows
BOUNDS = [0, 512, 1024, 2048, 3906]
NCH = len(BOUNDS) - 1
USPLIT = 1954         # u stream computed as two big subs (even start)
NACC = 2 * NCH + 1    # accT columns: (vw, qp) per chunk + final u

_cached = {}


def _build_program():
    """Hand-scheduled raw-bass program (no TileContext): full-width SBUF
    tensors, forward-RAW hazards only, handled with per-chunk DMA
    semaphores and one DVE->ACT semaphore."""
    import concourse.bass as bass
    import concourse.mybir as mybir

    f32 = mybir.dt.float32
    f16 = mybir.dt.float16
    DERF = mybir.ActivationFunctionType.Derivative_Erf
    nc = bass.Bass("TRN2", target_bir_lowering=False, debug=False,
                   num_devices=NCORES)
    d_ins, y_ins = [], []
    for j in range(NCH):
        a, e = BOUNDS[j], BOUNDS[j + 1]
        cw = e - a + 1        # chunks overlap by one column
        d_ins.append(nc.declare_dram_parameter(f"d{j}", [ROWS, cw], f16,
                                               isOutput=False))
        y_ins.append(nc.declare_dram_parameter(f"y{j}", [ROWS, cw], f16,
                                               isOutput=False))
    acc_out = nc.declare_dram_parameter("acc", [ROWS, NACC], f32,
                                        isOutput=True)

    from contextlib import ExitStack
    with ExitStack() as st:
        # one DMA-completion semaphore per chunk: its 32 increments can
        # only come from that chunk's two DMAs, so wait_ge(32) is exact
        dsem = [st.enter_context(nc.semaphore(f"dsem{j}"))
                for j in range(NCH)]
        v_sem = st.enter_context(nc.semaphore("v_sem"))
        out_sem = st.enter_context(nc.semaphore("out_sem"))
        dt = st.enter_context(nc.sbuf_tensor("dt", [ROWS, W], f16))
        yt = st.enter_context(nc.sbuf_tensor("yt", [ROWS, W], f16))
        df = st.enter_context(nc.sbuf_tensor("df", [ROWS, 5 * COLS], f16))
        sink = st.enter_context(nc.sbuf_tensor("sink", [ROWS, COLS], f16))
        accT = st.enter_context(nc.sbuf_tensor("accT", [ROWS, NACC], f32))

        # chunk 0 issued pre-Block on the two HWDGE rings so its data is
        # in flight while the Block-entry handshake runs
        e0 = BOUNDS[1]
        nc.sync.dma_start(dt[:, 0:e0 + 1], d_ins[0][:, :]) \
            .then_inc(dsem[0], 16)
        nc.scalar.dma_start(yt[:, 0:e0 + 1], y_ins[0][:, :]) \
            .then_inc(dsem[0], 16)

        block = st.enter_context(nc.Block())

        @block.sync
        def _(sync):
            # remaining d chunks on the SP HWDGE ring
            for j in range(1, NCH):
                a, e = BOUNDS[j], BOUNDS[j + 1]
                sync.dma_start(dt[:, a:e + 1], d_ins[j][:, :]) \
                    .then_inc(dsem[j], 16)

        @block.gpsimd
        def _(gpsimd):
            # remaining y chunks on the SWDGE ring, in parallel with d
            for j in range(1, NCH):
                a, e = BOUNDS[j], BOUNDS[j + 1]
                gpsimd.dma_start(yt[:, a:e + 1], y_ins[j][:, :]) \
                      .then_inc(dsem[j], 16)

        @block.vector
        def _(vector):
            for j in range(NCH):
                a, e = BOUNDS[j], BOUNDS[j + 1]
                vector.wait_ge(dsem[j], 32)
                # stream layout in df: [u | v | w | q | p]
                vector.tensor_sub(df[:, COLS + a:COLS + e],
                                  dt[:, a + 1:e + 1], yt[:, a:e]) \
                      .then_inc(v_sem, 1)
                vector.tensor_sub(df[:, 2 * COLS + a:2 * COLS + e],
                                  dt[:, a:e], yt[:, a + 1:e + 1]) \
                      .then_inc(v_sem, 1)
                vector.tensor_sub(df[:, 3 * COLS + a:3 * COLS + e],
                                  yt[:, a + 1:e + 1], yt[:, a:e]) \
                      .then_inc(v_sem, 1)
                vector.tensor_sub(df[:, 4 * COLS + a:4 * COLS + e],
                                  dt[:, a + 1:e + 1], dt[:, a:e]) \
                      .then_inc(v_sem, 1)
            # u as two big subs once everything is resident
            vector.tensor_sub(df[:, 0:USPLIT], dt[:, 0:USPLIT],
                              yt[:, 0:USPLIT]).then_inc(v_sem, 1)
            vector.tensor_sub(df[:, USPLIT:COLS], dt[:, USPLIT:COLS],
                              yt[:, USPLIT:COLS]).then_inc(v_sem, 1)

        @block.scalar
        def _(scalar):
            # warmup activation hoists the ~1.3us erf_derivative table
            # load off the critical path (garbage in, output discarded)
            scalar.activation(sink[:, 0:1], accT[:, 0:1], DERF,
                              bias=0.0, scale=SQRT_PI)

            def pair_act(base, a, cw, col):
                in_ap = bass.AP(df, base * COLS + a,
                                [[5 * COLS, ROWS], [COLS, 2], [1, cw]])
                out_ap = bass.AP(sink, 0,
                                 [[COLS, ROWS], [cw, 2], [1, cw]])
                scalar.activation(out_ap, in_ap, DERF, bias=0.0,
                                  scale=SQRT_PI,
                                  accum_out=accT[:, col:col + 1])

            for j in range(NCH):
                a, e = BOUNDS[j], BOUNDS[j + 1]
                cw = e - a
                # v,w ready after 2 subs; q,p after 4
                scalar.wait_ge(v_sem, 4 * j + 2)
                pair_act(1, a, cw, 2 * j)          # {v, w}
                scalar.wait_ge(v_sem, 4 * j + 4)
                pair_act(3, a, cw, 2 * j + 1)      # {q, p}
            # u: one full-width activation at the end
            scalar.wait_ge(v_sem, 4 * NCH + 2)
            scalar.activation(sink[:, 0:COLS], df[:, 0:COLS], DERF,
                              bias=0.0, scale=SQRT_PI,
                              accum_out=accT[:, NACC - 1:NACC])
            # flush the ACT datapath so the last accum lands in SBUF
            # before the DMA below reads accT
            scalar.drain()
            scalar.dma_start(acc_out[:, :], accT[:, :]).then_inc(out_sem, 16)
            # no wait on out_sem: the NEFF postamble outlasts the HBM
            # write receipt by 4x.  out_sem is never waited on, so its
            # monotonic growth across executions is harmless.
            # v_sem's final value proves all dsem increments delivered.
            for s in dsem:
                scalar.sem_clear(s)
            scalar.sem_clear(v_sem)

    return nc


def _overlap_tiles(x16):
    """[N] f16 -> per-core list of per-chunk contiguous [ROWS, cw+1]
    arrays."""
    sv = x16.strides[0]
    out = []
    for c in range(NCORES):
        base = x16[c * L:]
        m = np.lib.stride_tricks.as_strided(
            base, shape=(ROWS, W), strides=(COLS * sv, sv))
        chunks = []
        for j in range(NCH):
            a, e = BOUNDS[j], BOUNDS[j + 1]
            chunks.append(np.ascontiguousarray(m[:, a:e + 1]))
        out.append(chunks)
    return out


def make_in_maps(d, y):
    """Build the per-core input dicts from full fp32 d, y."""
    d16 = np.asarray(d, dtype=np.float16)
    y16 = np.asarray(y, dtype=np.float16)
    dts = _overlap_tiles(d16)
    yts = _overlap_tiles(y16)
    in_maps = []
    for c in range(NCORES):
        m = {}
        for j in range(NCH):
            m[f"d{j}"] = dts[c][j]
            m[f"y{j}"] = yts[c][j]
        in_maps.append(m)
    return in_maps


def _g64(t):
    t = np.asarray(t, dtype=np.float64)
    return np.exp(-np.pi * t * t)


def kernel(d, y):
    from concourse.bass_utils import run_bass_kernel_spmd

    d = np.ascontiguousarray(np.asarray(d, dtype=np.float32))
    y = np.ascontiguousarray(np.asarray(y, dtype=np.float32))

    if "nc" not in _cached:
        _cached["nc"] = _build_program()
    nc = _cached["nc"]

    in_maps = make_in_maps(d, y)
    if "warm" not in _cached:
        # first execution may see stale semaphore state left on the
        # device by other programs; it self-clears at its tail, so run
        # once and discard
        run_bass_kernel_spmd(nc, in_maps, list(range(NCORES)))
        _cached["warm"] = True
    res = run_bass_kernel_spmd(nc, in_maps, list(range(NCORES))).results

    # Device partial sums of DerivErf(sqrt(pi)*t) = (2/sqrt(pi)) g(t).
    # accT columns: [vw_0, qp_0, ..., vw_{NCH-1}, qp_{NCH-1}, u]
    acc = np.stack([r["acc"] for r in res]).astype(np.float64)  # [8,128,NACC]
    cols = acc.sum(axis=(0, 1)) * (SQRT_PI / 2.0)
    VW_dev = cols[0:2 * NCH:2].sum()
    QP_dev = cols[1:2 * NCH:2].sum()
    U_dev = cols[2 * NCH]

    d64 = d.astype(np.float64)
    y64 = y.astype(np.float64)
    cov = NCORES * L                                  # 3,999,744

    # tails in f64: u over j in [cov, D], others over k in [cov, D)
    jt = np.arange(cov, D + 1)
    U = U_dev + _g64(d64[jt] - y64[jt]).sum()
    kt = np.arange(cov, D)
    VW = VW_dev + _g64(d64[kt + 1] - y64[kt]).sum() \
        + _g64(d64[kt] - y64[kt + 1]).sum()
    QP = QP_dev + _g64(d64[kt + 1] - d64[kt]).sum() \
        + _g64(y64[kt + 1] - y64[kt]).sum()

    u0 = _g64(d64[0] - y64[0])
    uD = _g64(d64[D] - y64[D])
    # S1 + S2 - S3 = QP - VW - 2U + u0 + uD
    s12m3 = QP - VW - 2.0 * U + u0 + uD

    lsp32 = np.float32(0.5 * D * (math.log(2.0 * math.pi)
                                  + 2.0 * math.log(SIG)))
    total = math.exp(-float(lsp32)) * (D + s12m3 / 2.0)
    return np.array(total, dtype=np.float32)
